# revision 1
# baseline (speedup 1.0000x reference)
"""AFNO2D layer on 8 Trainium2 NeuronCores.

Sharding: channel-block parallel. C=768 = 8 blocks x 96 channels; the complex
MLP is block-diagonal over exactly these blocks, and the 2D FFT is independent
per channel — so core i handles channel block i end-to-end with zero
collectives.

Per-core pipeline (per batch b, all DFTs as dense matmuls on the 128x128 PE):
  S1  W-axis rfft, Hermitian-packed:  lhsT=Fpack[w,128] -> Ypack[fpack,(c,h)]
  TA  PE-transpose corner turn    -> Yt[h,(f,c)]
  S2  H-axis complex FFT (2 real matmuls + DVE combines) -> Zr,Zi [g,(f,c)]
  TB  PE-transpose                -> Zrt,Zit [c,(f,g)]
  L1  complex 96x96 matmul + bias + ReLU (PSUM accumulate pairs)
  L2  complex 96x96 matmul + bias; softshrink on DVE
  TC  PE-transpose                -> Or,Oi [g,(f,c)]
  S5  inverse H FFT + combines, packed -> ZIboth [h,(c,fpack)]
  TD  PE-transpose                -> ZIpack [fpack,(h,c)]
  S6  inverse W rfft (packed lhsT=Apack) + residual add -> out
Matmul dtype bf16 (PSUM accumulates f32); residual path f32. The output is
x + delta with ||delta||/||x|| ~ 0.05, so bf16 error in delta is attenuated
~20x in the final rel-err.

Shipped configuration is mode "full_deep_split": deeper sm/io tile pools
(bufs=4, so more MLP/S6 chunks pipeline across engines) plus a dedicated
2-bank PSUM pool for the corner-turn transposes so they don't contend with
matmul PSUM rotation. Marginal-rep measurements: 1.67 ms (base) -> 1.38 ms
(deep) -> 1.31 ms (deep+split).
"""

import sys

import numpy as np

try:
    import concourse  # noqa: F401
except ImportError:
    sys.path.insert(0, "/opt/trn_rl_repo")

import ml_dtypes

import concourse.bass as bass
import concourse.bacc as bacc
import concourse.mybir as mybir
import concourse.tile as tile
from concourse.bass_utils import run_bass_kernel_spmd

BF16 = ml_dtypes.bfloat16
DT = mybir.dt

B = 4
H = 128
W = 128
C = 96  # per-core channels (one MLP block)
F = 65  # rfft freqs along W
LAM = 0.01
N_CORES = 8

_CACHE = {}


def _host_matrices():
    """DFT matrices, all as matmul lhsT ([K, M]) layouts, bf16."""
    I = np.eye(W)
    R = np.fft.rfft(I, axis=1, norm="ortho")  # [w, f]: Y = x @ R
    fpack = np.concatenate([R.real, R.imag[:, 1:64]], axis=1)  # [w, 128]
    Dm = np.fft.fft(np.eye(H), axis=1, norm="ortho")  # [h, g]: Z = Y @ Dm
    DmI = np.fft.ifft(np.eye(H), axis=1, norm="ortho")  # [g, h]
    Ar = np.zeros((F, W))
    Ai = np.zeros((F, W))
    for f in range(F):
        e = np.zeros(F, dtype=complex)
        e[f] = 1.0
        Ar[f] = np.fft.irfft(e, n=W, norm="ortho")
        e = np.zeros(F, dtype=complex)
        e[f] = 1j
        Ai[f] = np.fft.irfft(e, n=W, norm="ortho")
    apack = np.concatenate([Ar, Ai[1:64]], axis=0)  # [fpack, w]
    c = lambda a: np.ascontiguousarray(a.astype(BF16))
    return {
        "fpack": c(fpack),
        "drt": c(Dm.real),
        "dit": c(Dm.imag),
        "ditn": c(-Dm.imag),
        "dirt": c(DmI.real),
        "diit": c(DmI.imag),
        "diitn": c(-DmI.imag),
        "apack": c(apack),
        "ident": c(np.eye(128)),
    }


def _build_nc_v5(reps=1, mode="v5"):
    """v5: zero-transpose pipeline. Every corner turn is fused into the
    adjacent DFT/MLP matmul by loading the DATA as the stationary operand
    (ldweights transposes it for free) and streaming the constant DFT/weight
    matrix as the moving operand:

      A  S1+TA : lhsT=x_c [w,h], rhs=fpack     -> psum [h, fpack] per c
      B  S2+TB : lhsT=Y_f [h,c], rhs=Dm        -> psum [c, g] per f
      L1       : standard (W1 stationary)      -> o1 [ci(+ones), (f,g)]
      D  L2+TC : lhsT=o1_f [ci+1,g], rhs=W2aug -> psum [g, co] per f
                 (bias rides the augmented ones-row; softshrink on eviction)
      E  S5+TD : lhsT=O_c [g,f], rhs=DI        -> psum [fpack, h] per c
                 (Re(0..64) parts 0..64, Im(1..63) parts 65..127 via
                  base-partition-64 matmuls; Im(0) slot overwritten by Re(64))
      F  S6    : standard (apack stationary) + bf16 residual reload
    """
    nc = bacc.Bacc(
        "TRN2", target_bir_lowering=False, debug=False, num_devices=N_CORES
    )

    def din(name, shape, dt):
        return nc.dram_tensor(name, shape, dt, kind="ExternalInput")

    x16 = din("x16", [B, W, C, H], DT.bfloat16)      # [w, c, h] per batch
    mats = {
        k: din(k, [128, 128], DT.bfloat16)
        for k in ["fpack", "drt", "dit", "ditn", "dirt", "diit", "diitn",
                  "apack"]
    }
    w1s = {k: din(k, [C, C], DT.bfloat16) for k in ["w1r", "w1i", "w1in"]}
    w2s = {k: din(k, [C + 1, C], DT.bfloat16)
           for k in ["w2r_br", "w2i_bi", "w2in0", "w2r0"]}
    bs = {k: din(k, [C, 1], DT.float32) for k in ["b1r", "b1i"]}
    nlam_d = din("nlam", [128, 1], DT.float32)
    out_ext = nc.dram_tensor("out", [B, H, W, C], DT.float32,
                             kind="ExternalOutput")

    CH = C * H          # 12288
    FG = F * 128        # 8320
    FC = F * C          # 6240
    RELU = mybir.ActivationFunctionType.Relu

    with tile.TileContext(nc) as tc:
        with (
            tc.tile_pool(name="const", bufs=1) as cpool,
            tc.tile_pool(name="big", bufs=4) as bpool,
            tc.tile_pool(name="zp", bufs=1) as zpool,
            tc.tile_pool(name="zt", bufs=2) as ztpool,
            tc.tile_pool(name="ot", bufs=2) as otpool,
            tc.tile_pool(name="o1", bufs=6) as o1pool,
            tc.tile_pool(name="sm", bufs=4) as smpool,
            tc.tile_pool(name="io", bufs=8) as iopool,
            tc.tile_pool(name="ps", bufs=5, space="PSUM") as pspool,
            tc.tile_pool(name="psf", bufs=3, space="PSUM") as psfpool,
        ):
            M = {}
            for k in mats:
                M[k] = cpool.tile([128, 128], DT.bfloat16, tag=f"m_{k}",
                                  name=f"m_{k}")
                nc.sync.dma_start(M[k][:], mats[k][:])
            Wt = {}
            for k in w1s:
                Wt[k] = cpool.tile([C, C], DT.bfloat16, tag=f"w_{k}",
                                   name=f"wt_{k}")
                nc.sync.dma_start(Wt[k][:], w1s[k][:])
            W2t = {}
            for k in w2s:
                W2t[k] = cpool.tile([C + 1, C], DT.bfloat16, tag=f"w_{k}",
                                    name=f"wt_{k}")
                nc.sync.dma_start(W2t[k][:], w2s[k][:])
            Bt = {}
            for k in bs:
                Bt[k] = cpool.tile([C, 1], DT.float32, tag=f"b_{k}",
                                   name=f"bt_{k}")
                nc.sync.dma_start(Bt[k][:], bs[k][:])
            nlam = cpool.tile([128, 1], DT.float32, tag="b_nlam", name="nlam")
            nc.sync.dma_start(nlam[:], nlam_d[:])

            for b in [bb for _ in range(reps) for bb in range(B)]:
                xb = bpool.tile([128, CH], DT.bfloat16, tag="big", name="xb")
                nc.sync.dma_start(xb[:], x16[b].rearrange("w c h -> w (c h)"))
                xbv = xb[:].rearrange("p (c h) -> p c h", h=H)

                # ---- A: S1 + TA fused -> yt [h, (fpack, c)] ----
                yt = bpool.tile([128, CH], DT.bfloat16, tag="big", name="yt")
                ytv = yt[:].rearrange("p (f c) -> p c f", c=C)
                for c0 in range(0, C, 4):
                    psA = pspool.tile([128, 512], DT.float32, tag="ps",
                                      name="psA")
                    for j in range(4):
                        nc.tensor.matmul(psA[:, 128 * j:128 * (j + 1)],
                                         xbv[:, c0 + j, :], M["fpack"][:],
                                         start=True, stop=True)
                    ev = nc.scalar.copy if (c0 // 4) % 3 != 2 else \
                        nc.vector.tensor_copy
                    ev(ytv[:, c0:c0 + 4, :],
                       psA[:].rearrange("p (j f) -> p j f", j=4))

                # ---- B: S2 + TB fused -> zrt/zit [c, (f, g)] ----
                zrt = ztpool.tile([C, FG], DT.bfloat16, tag="zt", name="zrt")
                zit = ztpool.tile([C, FG], DT.bfloat16, tag="zt", name="zit")
                for f0 in range(0, 68, 4):
                    fs = list(range(f0, min(f0 + 4, F)))
                    if not fs:
                        break
                    n = 128 * len(fs)
                    psR = pspool.tile([C, 512], DT.float32, tag="ps",
                                      name="psR")
                    psI = pspool.tile([C, 512], DT.float32, tag="ps",
                                      name="psI")
                    for j, f in enumerate(fs):
                        pr = psR[:, 128 * j:128 * j + 128]
                        pi = psI[:, 128 * j:128 * j + 128]
                        lr = yt[:, 96 * f:96 * (f + 1)]
                        edge = f in (0, 64)
                        nc.tensor.matmul(pr, lr, M["drt"][:],
                                         start=True, stop=edge)
                        nc.tensor.matmul(pi, lr, M["dit"][:],
                                         start=True, stop=edge)
                        if not edge:
                            li = yt[:, 96 * (64 + f):96 * (65 + f)]
                            nc.tensor.matmul(pr, li, M["ditn"][:],
                                             start=False, stop=True)
                            nc.tensor.matmul(pi, li, M["drt"][:],
                                             start=False, stop=True)
                    nc.scalar.copy(zrt[:, 128 * f0:128 * f0 + n], psR[:, :n])
                    nc.vector.tensor_copy(zit[:, 128 * f0:128 * f0 + n],
                                          psI[:, :n])

                # ---- L1 (chunk k) + L2+TC (group k-1) interleaved ----
                or_ = otpool.tile([128, FC], DT.bfloat16, tag="ot", name="or_")
                oi_ = otpool.tile([128, FC], DT.bfloat16, tag="ot", name="oi_")
                o1rs, o1is = [], []

                def l1_chunk(k):
                    lo = 512 * k
                    hi = min(lo + 512, FG)
                    n = hi - lo
                    psr = pspool.tile([C, 512], DT.float32, tag="ps",
                                      name="psL1r")
                    psi = pspool.tile([C, 512], DT.float32, tag="ps",
                                      name="psL1i")
                    nc.tensor.matmul(psr[:, :n], Wt["w1r"][:], zrt[:, lo:hi],
                                     start=True, stop=False)
                    nc.tensor.matmul(psi[:, :n], Wt["w1r"][:], zit[:, lo:hi],
                                     start=True, stop=False)
                    nc.tensor.matmul(psr[:, :n], Wt["w1in"][:], zit[:, lo:hi],
                                     start=False, stop=True)
                    nc.tensor.matmul(psi[:, :n], Wt["w1i"][:], zrt[:, lo:hi],
                                     start=False, stop=True)
                    o1r = o1pool.tile([C + 1, 512], DT.bfloat16, tag="o1",
                                      name="o1r")
                    o1i = o1pool.tile([C + 1, 512], DT.bfloat16, tag="o1",
                                      name="o1i")
                    nc.scalar.activation(o1r[0:C, :n], psr[:, :n], RELU,
                                         bias=Bt["b1r"][:, 0:1])
                    nc.vector.tensor_scalar(
                        o1i[0:C, :n], psi[:, :n], Bt["b1i"][:, 0:1], 0.0,
                        mybir.AluOpType.add, mybir.AluOpType.max)
                    nc.gpsimd.memset(o1r[C:C + 1, :n], 1.0)
                    nc.gpsimd.memset(o1i[C:C + 1, :n], 1.0)
                    o1rs.append(o1r)
                    o1is.append(o1i)

                def l2tc_group(k):
                    fs = list(range(4 * k, min(4 * k + 4, F)))
                    nf = len(fs)
                    n = 96 * nf
                    psor = pspool.tile([128, 384], DT.float32, tag="ps",
                                       name="psor")
                    psoi = pspool.tile([128, 384], DT.float32, tag="ps",
                                       name="psoi")
                    for j in range(nf):
                        lr = o1rs[k][0:C + 1, 128 * j:128 * j + 128]
                        li = o1is[k][0:C + 1, 128 * j:128 * j + 128]
                        po = psor[:, 96 * j:96 * j + 96]
                        poi = psoi[:, 96 * j:96 * j + 96]
                        nc.tensor.matmul(po, lr, W2t["w2r_br"][:],
                                         start=True, stop=False)
                        nc.tensor.matmul(poi, lr, W2t["w2i_bi"][:],
                                         start=True, stop=False)
                        nc.tensor.matmul(po, li, W2t["w2in0"][:],
                                         start=False, stop=True)
                        nc.tensor.matmul(poi, li, W2t["w2r0"][:],
                                         start=False, stop=True)
                    for ps, dst in [(psor, or_), (psoi, oi_)]:
                        # softshrink(u) = relu(u - lam) + min(u + lam, 0)
                        a1 = smpool.tile([128, 384], DT.bfloat16, tag="sm1",
                                         name="a1")
                        a2 = smpool.tile([128, 384], DT.bfloat16, tag="sm2",
                                         name="a2")
                        nc.scalar.activation(a1[:, :n], ps[:, :n], RELU,
                                             bias=nlam[:, 0:1])
                        nc.vector.tensor_scalar(
                            a2[:, :n], ps[:, :n], LAM, 0.0,
                            mybir.AluOpType.add, mybir.AluOpType.min)
                        nc.gpsimd.tensor_add(
                            dst[:, 96 * 4 * k:96 * 4 * k + n],
                            a1[:, :n], a2[:, :n])

                l1_chunk(0)
                for k in range(1, 17):
                    l1_chunk(k)
                    l2tc_group(k - 1)
                l2tc_group(16)

                # ---- E: S5 + TD fused -> zp [fpack, (h, c)] ----
                zp = zpool.tile([128, CH], DT.bfloat16, tag="zp", name="zp")
                zpv = zp[:].rearrange("p (h c) -> p c h", c=C)
                orv = or_[:].rearrange("p (f c) -> p c f", c=C)
                oiv = oi_[:].rearrange("p (f c) -> p c f", c=C)
                for c0 in range(0, C, 4):
                    ps5 = pspool.tile([128, 512], DT.float32, tag="ps",
                                      name="ps5")
                    for j in range(4):
                        c = c0 + j
                        sl = slice(128 * j, 128 * j + 128)
                        # Im block first: parts 64..127 = Im(f=0..63)
                        nc.tensor.matmul(ps5[64:128, sl], orv[:, c, 0:64],
                                         M["diit"][:], start=True, stop=False)
                        nc.tensor.matmul(ps5[64:128, sl], oiv[:, c, 0:64],
                                         M["dirt"][:], start=False, stop=True)
                        # Re block: parts 0..64 = Re(f=0..64); overwrites
                        # the Im(0) garbage on part 64 with Re(64)
                        nc.tensor.matmul(ps5[0:65, sl], orv[:, c, 0:65],
                                         M["dirt"][:], start=True, stop=False)
                        nc.tensor.matmul(ps5[0:65, sl], oiv[:, c, 0:65],
                                         M["diitn"][:], start=False, stop=True)
                    nc.scalar.copy(zpv[:, c0:c0 + 4, :],
                                   ps5[:].rearrange("p (j h) -> p j h", j=4))

                # ---- F: S6 + bf16 residual from xb ----
                outv = out_ext[b].rearrange("h w c -> w h c")
                xbr = xb[:].rearrange("p (c h) -> p h c", h=H)
                for k in range(26):
                    lo = 480 * k
                    hi = min(lo + 480, CH)
                    n = hi - lo
                    h0, h1 = lo // C, hi // C
                    ps6 = psfpool.tile([128, 480], DT.float32, tag="psf",
                                       name="ps6")
                    nc.tensor.matmul(ps6[:, :n], M["apack"][:], zp[:, lo:hi],
                                     start=True, stop=True)
                    oc = iopool.tile([128, 480], DT.float32, tag="oc",
                                     name="oc")
                    nc.vector.tensor_add(
                        oc[:, :n].rearrange("w (h c) -> w h c", c=C),
                        ps6[:, :n].rearrange("w (h c) -> w h c", c=C),
                        xbr[:, h0:h1, :])
                    dmae = nc.sync if k % 2 == 0 else nc.gpsimd
                    dmae.dma_start(
                        outv[:, h0:h1, :],
                        oc[:, :n].rearrange("w (h c) -> w h c", c=C))

    nc.compile()
    return nc


def _build_nc(reps=1, mode="full"):
    if mode == "v4":
        mode = "full_s1ta"
    if mode.startswith("v5"):
        return _build_nc_v5(reps, mode)
    if mode.startswith("v2"):
        return _build_nc_v2(reps, mode)
    if mode.startswith("v3"):
        return _build_nc_v3(reps, mode)
    if mode == "tiny":
        return _build_nc_tiny(reps)
    deep = "deep" in mode
    wide = "wide" in mode
    split = "split" in mode
    nc = bacc.Bacc(
        "TRN2", target_bir_lowering=False, debug=False, num_devices=N_CORES
    )

    def din(name, shape, dt):
        return nc.dram_tensor(name, shape, dt, kind="ExternalInput")

    x16 = din("x16", [B, W, C, H], DT.bfloat16)
    xres = din("xres", [B, H, W, C], DT.float32)
    mats = {
        k: din(k, [128, 128], DT.bfloat16)
        for k in ["fpack", "drt", "dit", "ditn", "dirt", "diit", "diitn", "apack", "ident"]
    }
    wts = {k: din(k, [C, C], DT.bfloat16)
           for k in ["w1r", "w1i", "w1in", "w2r", "w2i", "w2in"]}
    bs = {k: din(k, [C, 1], DT.float32) for k in ["b1r", "b1i", "b2r", "b2i"]}
    out_ext = nc.dram_tensor("out", [B, H, W, C], DT.float32, kind="ExternalOutput")

    CH = C * H          # 12288
    FC = F * C          # 6240
    FG = F * 128        # 8320

    with tile.TileContext(nc) as tc:
        with (
            tc.tile_pool(name="const", bufs=1) as cpool,
            tc.tile_pool(name="big", bufs=3) as bpool,
            tc.tile_pool(name="pa", bufs=2 if deep else 3) as papool,
            tc.tile_pool(name="pb", bufs=4) as pbpool,
            tc.tile_pool(name="sm", bufs=4 if deep else 2) as spool,
            tc.tile_pool(name="io", bufs=4 if deep else 2) as iopool,
            tc.tile_pool(name="ps", bufs=(5 if "s53" in mode else 6) if split
                         else 8, space="PSUM") as pspool,
            tc.tile_pool(name="ps2", bufs=3 if "s53" in mode else 2,
                         space="PSUM") as pstpool2,
        ):
            tag_t = "pst" if split else "ps"
            pool_t = pstpool2 if split else pspool
            M = {}
            for k in mats:
                M[k] = cpool.tile([128, 128], DT.bfloat16, tag=f"m_{k}", name=f"m_{k}")
                nc.sync.dma_start(M[k][:], mats[k][:])
            Wt = {}
            for k in wts:
                Wt[k] = cpool.tile([C, C], DT.bfloat16, tag=f"w_{k}", name=f"wt_{k}")
                nc.sync.dma_start(Wt[k][:], wts[k][:])
            Bt = {}
            for k in bs:
                Bt[k] = cpool.tile([C, 1], DT.float32, tag=f"b_{k}", name=f"bt_{k}")
                nc.sync.dma_start(Bt[k][:], bs[k][:])
            ID = M["ident"]

            for b in [bb for _ in range(reps) for bb in range(B)]:
                # ---- load x (pre-packed bf16 [w, (c, h)]) ----
                xb = bpool.tile([128, CH], DT.bfloat16, tag="big")
                nc.sync.dma_start(xb[:], x16[b].rearrange("w c h -> w (c h)"))

                if mode == "full_s1ta":
                    # S1+TA fused: x-slice as lhsT (ldweights transposes it),
                    # out psum = [h, fpack] per c directly
                    yt = bpool.tile([128, CH], DT.bfloat16, tag="big")
                    xbv = xb[:].rearrange("p (c h) -> p c h", h=H)
                    ytv = yt[:].rearrange("p (f c) -> p c f", c=C)
                    for c0 in range(0, C, 4):
                        psF = pspool.tile([128, 512], DT.float32, tag="ps")
                        for j in range(4):
                            nc.tensor.matmul(
                                psF[:, 128 * j:128 * (j + 1)],
                                xbv[:, c0 + j, :], M["fpack"][:],
                                start=True, stop=True)
                        nc.scalar.copy(ytv[:, c0:c0 + 4, :], psF[:])
                else:
                    # ---- S1: W-rfft packed ----
                    yp = bpool.tile([128, CH], DT.bfloat16, tag="big")
                    for k in range(CH // 512):
                        s = slice(512 * k, 512 * (k + 1))
                        ps = pspool.tile([128, 512], DT.float32, tag="ps")
                        nc.tensor.matmul(ps[:], M["fpack"][:], xb[:, s],
                                         start=True, stop=True)
                        nc.scalar.copy(yp[:, s], ps[:])

                    # ---- TA: [fpack,(c,h)] -> Yt [h,(f,c)] ----
                    yt = bpool.tile([128, CH], DT.bfloat16, tag="big")
                    ypv = yp[:].rearrange("p (c h) -> p c h", h=H)
                    ytv = yt[:].rearrange("p (f c) -> p c f", c=C)
                    gA = 8 if wide else 4
                    for c0 in range(0, C, gA):
                        psT = pool_t.tile([128, 128 * gA], DT.bfloat16, tag=tag_t, name="psT")
                        for j in range(gA):
                            nc.tensor.transpose(
                                psT[:, 128 * j:128 * (j + 1)], ypv[:, c0 + j, :], ID[:]
                            )
                        nc.scalar.copy(ytv[:, c0:c0 + gA, :], psT[:])

                # ---- S2: H-axis complex FFT (PE-accumulated combines) ----
                zr = papool.tile([128, FC], DT.bfloat16, tag="pa")
                zi = papool.tile([128, FC], DT.bfloat16, tag="pa")
                # edges f=0 (cols 0:96) and f=64 (cols 6144:6240): Yi=0 there
                for cols in [slice(0, 96), slice(6144, 6240)]:
                    pe1 = pspool.tile([128, 96], DT.float32, tag="ps")
                    pe2 = pspool.tile([128, 96], DT.float32, tag="ps")
                    nc.tensor.matmul(pe1[:], M["drt"][:], yt[:, cols],
                                     start=True, stop=True)
                    nc.tensor.matmul(pe2[:], M["dit"][:], yt[:, cols],
                                     start=True, stop=True)
                    nc.scalar.copy(zr[:, cols], pe1[:])
                    nc.scalar.copy(zi[:, cols], pe2[:])
                for j in range(12):
                    sa = slice(96 + 504 * j, 96 + 504 * (j + 1))      # fr f=1..63
                    sb = slice(6240 + 504 * j, 6240 + 504 * (j + 1))  # fi f=1..63
                    pszr = pspool.tile([128, 504], DT.float32, tag="ps")
                    pszi = pspool.tile([128, 504], DT.float32, tag="ps")
                    nc.tensor.matmul(pszr[:], M["drt"][:], yt[:, sa], start=True, stop=False)
                    nc.tensor.matmul(pszr[:], M["ditn"][:], yt[:, sb], start=False, stop=True)
                    nc.tensor.matmul(pszi[:], M["drt"][:], yt[:, sb], start=True, stop=False)
                    nc.tensor.matmul(pszi[:], M["dit"][:], yt[:, sa], start=False, stop=True)
                    nc.vector.tensor_copy(zr[:, sa], pszr[:])
                    nc.vector.tensor_copy(zi[:, sa], pszi[:])

                # ---- TB: [g,(f,c)] -> [c,(f,g)] ----
                zrt = pbpool.tile([C, FG], DT.bfloat16, tag="pb")
                zit = pbpool.tile([C, FG], DT.bfloat16, tag="pb")
                gB = 8 if wide else 4
                for (src, dst) in [(zr, zrt), (zi, zit)]:
                    sv = src[:].rearrange("p (f c) -> p f c", c=C)
                    for f0 in range(0, 64, gB):
                        psB = pool_t.tile([C, 128 * gB], DT.bfloat16, tag=tag_t, name="psB")
                        for j in range(gB):
                            nc.tensor.transpose(
                                psB[:, 128 * j:128 * (j + 1)], sv[:, f0 + j, :], ID[:]
                            )
                        nc.scalar.copy(dst[:, 128 * f0:128 * (f0 + gB)], psB[:])
                    psB = pool_t.tile([C, 128], DT.bfloat16, tag=tag_t, name="psB")
                    nc.tensor.transpose(psB[:], sv[:, 64, :], ID[:])
                    nc.scalar.copy(dst[:, 128 * 64:128 * 65], psB[:])

                # ---- L1 + L2 MLP fused (chunks over (f,g)) ----
                o2r = pbpool.tile([C, FG], DT.bfloat16, tag="pb")
                o2i = pbpool.tile([C, FG], DT.bfloat16, tag="pb")
                chunks = [slice(512 * k, min(512 * (k + 1), FG))
                          for k in range((FG + 511) // 512)]
                for s in chunks:
                    n = s.stop - s.start
                    psr = pspool.tile([C, n], DT.float32, tag="ps")
                    psi = pspool.tile([C, n], DT.float32, tag="ps")
                    nc.tensor.matmul(psr[:], Wt["w1r"][:], zrt[:, s], start=True, stop=False)
                    nc.tensor.matmul(psr[:], Wt["w1in"][:], zit[:, s], start=False, stop=True)
                    nc.tensor.matmul(psi[:], Wt["w1r"][:], zit[:, s], start=True, stop=False)
                    nc.tensor.matmul(psi[:], Wt["w1i"][:], zrt[:, s], start=False, stop=True)
                    o1rc = spool.tile([C, 512], DT.bfloat16, tag="o1r", name="o1rc")
                    o1ic = spool.tile([C, 512], DT.bfloat16, tag="o1i", name="o1ic")
                    nc.scalar.activation(o1rc[:, :n], psr[:],
                                         mybir.ActivationFunctionType.Relu,
                                         bias=Bt["b1r"][:, 0:1])
                    nc.scalar.activation(o1ic[:, :n], psi[:],
                                         mybir.ActivationFunctionType.Relu,
                                         bias=Bt["b1i"][:, 0:1])
                    psr2 = pspool.tile([C, n], DT.float32, tag="ps")
                    psi2 = pspool.tile([C, n], DT.float32, tag="ps")
                    nc.tensor.matmul(psr2[:], Wt["w2r"][:], o1rc[:, :n], start=True, stop=False)
                    nc.tensor.matmul(psr2[:], Wt["w2in"][:], o1ic[:, :n], start=False, stop=True)
                    nc.tensor.matmul(psi2[:], Wt["w2r"][:], o1ic[:, :n], start=True, stop=False)
                    nc.tensor.matmul(psi2[:], Wt["w2i"][:], o1rc[:, :n], start=False, stop=True)
                    for ps, bias, dst in [(psr2, "b2r", o2r), (psi2, "b2i", o2i)]:
                        t = spool.tile([C, 512], DT.bfloat16, tag="ss1", name="sst")
                        tcl = spool.tile([C, 512], DT.bfloat16, tag="ss2", name="sscl")
                        nc.scalar.activation(t[:, :n], ps[:],
                                             mybir.ActivationFunctionType.Identity,
                                             bias=Bt[bias][:, 0:1])
                        nc.vector.tensor_scalar(
                            tcl[:, :n], t[:, :n], -LAM, LAM,
                            mybir.AluOpType.max, mybir.AluOpType.min)
                        nc.vector.tensor_sub(dst[:, s], t[:, :n], tcl[:, :n])

                # ---- TC: [c,(f,g)] -> [g,(f,c)] ----
                or_ = papool.tile([128, FC], DT.bfloat16, tag="pa")
                oi_ = papool.tile([128, FC], DT.bfloat16, tag="pa")
                for (src, dst) in [(o2r, or_), (o2i, oi_)]:
                    sv = src[:].rearrange("p (f g) -> p f g", g=128)
                    if wide:
                        for k in range(7):
                            f0 = 10 * k
                            nf = min(10, F - f0)
                            psC = pool_t.tile([128, 960], DT.bfloat16, tag=tag_t, name="psC")
                            for j in range(nf):
                                nc.tensor.transpose(
                                    psC[:, 96 * j:96 * (j + 1)], sv[:, f0 + j, :],
                                    ID[0:96, 0:96]
                                )
                            nc.scalar.copy(dst[:, 96 * f0:96 * (f0 + nf)],
                                           psC[:, :96 * nf])
                    else:
                        for f0 in range(0, F, 5):
                            psC = pool_t.tile([128, 480], DT.bfloat16, tag=tag_t, name="psC")
                            for j in range(5):
                                nc.tensor.transpose(
                                    psC[:, 96 * j:96 * (j + 1)], sv[:, f0 + j, :],
                                    ID[0:96, 0:96]
                                )
                            nc.scalar.copy(dst[:, 96 * f0:96 * (f0 + 5)], psC[:])

                # ---- S5: inverse H FFT (PE-accumulated), packed output ----
                zb = bpool.tile([128, CH], DT.bfloat16, tag="big")  # [h,(c,fpack)]
                zbv = zb[:].rearrange("p (c f) -> p f c", f=128)
                for j in range(13):
                    s = slice(480 * j, 480 * (j + 1))
                    pszr = pspool.tile([128, 480], DT.float32, tag="ps")
                    pszi = pspool.tile([128, 480], DT.float32, tag="ps")
                    nc.tensor.matmul(pszr[:], M["dirt"][:], or_[:, s], start=True, stop=False)
                    nc.tensor.matmul(pszr[:], M["diitn"][:], oi_[:, s], start=False, stop=True)
                    nc.tensor.matmul(pszi[:], M["dirt"][:], oi_[:, s], start=True, stop=False)
                    nc.tensor.matmul(pszi[:], M["diit"][:], or_[:, s], start=False, stop=True)
                    f0 = 5 * j
                    nc.vector.tensor_copy(zbv[:, f0:f0 + 5, :], pszr[:])
                    # imag part -> fpack rows 64+f, dropping f=0 and f=64
                    if j == 0:
                        nc.vector.tensor_copy(zbv[:, 65:69, :], pszi[:, 96:480])
                    elif j == 12:
                        nc.vector.tensor_copy(zbv[:, 124:128, :], pszi[:, 0:384])
                    else:
                        nc.vector.tensor_copy(zbv[:, 64 + f0:69 + f0, :], pszi[:])

                # ---- TD: [h,(c,fpack)] -> ZIpack [fpack,(h,c)] ----
                zp = bpool.tile([128, CH], DT.bfloat16, tag="big")
                zbc = zb[:].rearrange("p (c f) -> p c f", f=128)
                zpv = zp[:].rearrange("p (h c) -> p c h", c=C)
                gD = 8 if wide else 4
                for c0 in range(0, C, gD):
                    psD = pool_t.tile([128, 128 * gD], DT.bfloat16, tag=tag_t, name="psD")
                    for j in range(gD):
                        nc.tensor.transpose(
                            psD[:, 128 * j:128 * (j + 1)], zbc[:, c0 + j, :], ID[:]
                        )
                    nc.scalar.copy(zpv[:, c0:c0 + gD, :], psD[:])

                # ---- S6: inverse W rfft + residual ----
                xrv = xres[b].rearrange("h w c -> w h c")
                orv = out_ext[b].rearrange("h w c -> w h c")
                xbv = xb[:].rearrange("p (c h) -> p h c", h=H)
                for j in range(26):
                    lo = 480 * j
                    hi = min(lo + 480, CH)
                    n = hi - lo
                    h0, h1 = lo // C, hi // C
                    ps6 = pspool.tile([128, n], DT.float32, tag="ps")
                    nc.tensor.matmul(ps6[:], M["apack"][:], zp[:, lo:hi],
                                     start=True, stop=True)
                    oc = iopool.tile([128, 480], DT.float32, tag="oc")
                    dmae = nc.gpsimd if "gp" in mode else nc.sync
                    if mode.startswith("full"):
                        xr = iopool.tile([128, 480], DT.float32, tag="xr")
                        dmae.dma_start(
                            xr[:, :n].rearrange("w (h c) -> w h c", c=C),
                            xrv[:, h0:h1, :])
                        nc.vector.tensor_add(oc[:, :n], ps6[:], xr[:, :n])
                    else:
                        nc.vector.tensor_add(
                            oc[:, :n].rearrange("w (h c) -> w h c", c=C),
                            ps6[:].rearrange("w (h c) -> w h c", c=C),
                            xbv[:, h0:h1, :])
                    if mode != "nos6dma":
                        dmae.dma_start(orv[:, h0:h1, :],
                                       oc[:, :n].rearrange("w (h c) -> w h c", c=C))

    nc.compile()
    return nc


def _build_nc_v3(reps=1, mode="v3"):
    noturns = "noturns" in mode
    """v3 = v1 structure (PE turns) with:
    - softshrink as relu(u-lam) - relu(-u-lam), folded into the ACT bias
      (kills the slow dual-scalar DVE ops)
    - wider bf16 PSUM tiles for turn evictions (fewer, bigger ACT/DVE ops)
    - eviction work split across ACT and DVE
    """
    nc = bacc.Bacc(
        "TRN2", target_bir_lowering=False, debug=False, num_devices=N_CORES
    )

    def din(name, shape, dt):
        return nc.dram_tensor(name, shape, dt, kind="ExternalInput")

    x16 = din("x16", [B, W, C, H], DT.bfloat16)
    xres = din("xres", [B, H, W, C], DT.float32)
    mats = {
        k: din(k, [128, 128], DT.bfloat16)
        for k in ["fpack", "drt", "dit", "ditn", "dirt", "diit", "diitn",
                  "apack", "ident"]
    }
    wts = {k: din(k, [C, C], DT.bfloat16)
           for k in ["w1r", "w1i", "w1in", "w2r", "w2i", "w2in"]}
    bias_names = ["b1r", "b1i", "b2rm", "b2rp", "b2im", "b2ip"]
    bs = {k: din(k, [C, 1], DT.float32) for k in bias_names}
    out_ext = nc.dram_tensor("out", [B, H, W, C], DT.float32, kind="ExternalOutput")

    CH = C * H
    FC = F * C
    FG = F * 128

    with tile.TileContext(nc) as tc:
        with (
            tc.tile_pool(name="const", bufs=1) as cpool,
            tc.tile_pool(name="big", bufs=3) as bpool,
            tc.tile_pool(name="pa", bufs=3) as papool,
            tc.tile_pool(name="pb", bufs=4) as pbpool,
            tc.tile_pool(name="sm", bufs=2) as spool,
            tc.tile_pool(name="io", bufs=2) as iopool,
            tc.tile_pool(name="ps", bufs=6, space="PSUM") as pspool,
            tc.tile_pool(name="pst", bufs=2, space="PSUM") as pstpool,
        ):
            M = {}
            for k in mats:
                M[k] = cpool.tile([128, 128], DT.bfloat16, tag=f"m_{k}", name=f"m_{k}")
                nc.sync.dma_start(M[k][:], mats[k][:])
            Wt = {}
            for k in wts:
                Wt[k] = cpool.tile([C, C], DT.bfloat16, tag=f"w_{k}", name=f"wt_{k}")
                nc.sync.dma_start(Wt[k][:], wts[k][:])
            Bt = {}
            for k in bs:
                Bt[k] = cpool.tile([C, 1], DT.float32, tag=f"b_{k}", name=f"bt_{k}")
                nc.sync.dma_start(Bt[k][:], bs[k][:])
            ID = M["ident"]

            for b in [bb for _ in range(reps) for bb in range(B)]:
                xb = bpool.tile([128, CH], DT.bfloat16, tag="big")
                nc.sync.dma_start(xb[:], x16[b].rearrange("w c h -> w (c h)"))

                # ---- S1: W-rfft packed; evict on DVE ----
                yp = bpool.tile([128, CH], DT.bfloat16, tag="big")
                for k in range(CH // 512):
                    s = slice(512 * k, 512 * (k + 1))
                    ps = pspool.tile([128, 512], DT.float32, tag="ps")
                    nc.tensor.matmul(ps[:], M["fpack"][:], xb[:, s],
                                     start=True, stop=True)
                    nc.vector.tensor_copy(yp[:, s], ps[:])

                # ---- TA: [fpack,(c,h)] -> Yt [h,(f,c)]; 8-c groups ----
                yt = bpool.tile([128, CH], DT.bfloat16, tag="big")
                ypv = yp[:].rearrange("p (c h) -> p c h", h=H)
                ytv = yt[:].rearrange("p (f c) -> p c f", c=C)
                if noturns:
                    for k in range(CH // 1024):
                        s = slice(1024 * k, 1024 * (k + 1))
                        nc.vector.tensor_copy(yt[:, s], yp[:, s])
                else:
                    for c0 in range(0, C, 8):
                        psT = pstpool.tile([128, 1024], DT.bfloat16, tag="pst")
                        for j in range(8):
                            nc.tensor.transpose(
                                psT[:, 128 * j:128 * (j + 1)], ypv[:, c0 + j, :], ID[:]
                            )
                        nc.vector.tensor_copy(ytv[:, c0:c0 + 8, :], psT[:])

                # ---- S2: H-axis complex FFT; evicts on DVE ----
                zr = papool.tile([128, FC], DT.bfloat16, tag="pa")
                zi = papool.tile([128, FC], DT.bfloat16, tag="pa")
                for cols in [slice(0, 96), slice(6144, 6240)]:
                    pe1 = pspool.tile([128, 96], DT.float32, tag="ps")
                    pe2 = pspool.tile([128, 96], DT.float32, tag="ps")
                    nc.tensor.matmul(pe1[:], M["drt"][:], yt[:, cols],
                                     start=True, stop=True)
                    nc.tensor.matmul(pe2[:], M["dit"][:], yt[:, cols],
                                     start=True, stop=True)
                    nc.vector.tensor_copy(zr[:, cols], pe1[:])
                    nc.vector.tensor_copy(zi[:, cols], pe2[:])
                for j in range(12):
                    sa = slice(96 + 504 * j, 96 + 504 * (j + 1))
                    sb = slice(6240 + 504 * j, 6240 + 504 * (j + 1))
                    pszr = pspool.tile([128, 504], DT.float32, tag="ps")
                    pszi = pspool.tile([128, 504], DT.float32, tag="ps")
                    nc.tensor.matmul(pszr[:], M["drt"][:], yt[:, sa], start=True, stop=False)
                    nc.tensor.matmul(pszr[:], M["ditn"][:], yt[:, sb], start=False, stop=True)
                    nc.tensor.matmul(pszi[:], M["drt"][:], yt[:, sb], start=True, stop=False)
                    nc.tensor.matmul(pszi[:], M["dit"][:], yt[:, sa], start=False, stop=True)
                    nc.vector.tensor_copy(zr[:, sa], pszr[:])
                    nc.vector.tensor_copy(zi[:, sa], pszi[:])

                # ---- TB: [g,(f,c)] -> [c,(f,g)]; 8-f groups; evict ACT ----
                zrt = pbpool.tile([C, FG], DT.bfloat16, tag="pb")
                zit = pbpool.tile([C, FG], DT.bfloat16, tag="pb")
                for (src, dst) in [(zr, zrt), (zi, zit)]:
                    sv = src[:].rearrange("p (f c) -> p f c", c=C)
                    if noturns:
                        for k in range(6):
                            s = slice(1024 * k, min(1024 * (k + 1), FC))
                            nc.scalar.copy(dst[0:96, s], src[0:96, s])
                        nc.scalar.copy(dst[0:96, FC:FG], dst[0:96, 0:FG - FC])
                        continue
                    for f0 in range(0, 64, 8):
                        psB = pstpool.tile([C, 1024], DT.bfloat16, tag="pst")
                        for j in range(8):
                            nc.tensor.transpose(
                                psB[:, 128 * j:128 * (j + 1)], sv[:, f0 + j, :], ID[:]
                            )
                        nc.scalar.copy(dst[:, 128 * f0:128 * (f0 + 8)], psB[:])
                    psB = pstpool.tile([C, 128], DT.bfloat16, tag="pst")
                    nc.tensor.transpose(psB[:], sv[:, 64, :], ID[:])
                    nc.scalar.copy(dst[:, 128 * 64:128 * 65], psB[:])

                # ---- L1 + L2 MLP fused; softshrink via two ReLUs ----
                o2r = pbpool.tile([C, FG], DT.bfloat16, tag="pb")
                o2i = pbpool.tile([C, FG], DT.bfloat16, tag="pb")
                chunks = [slice(512 * k, min(512 * (k + 1), FG))
                          for k in range((FG + 511) // 512)]
                for s in chunks:
                    n = s.stop - s.start
                    psr = pspool.tile([C, n], DT.float32, tag="ps")
                    psi = pspool.tile([C, n], DT.float32, tag="ps")
                    nc.tensor.matmul(psr[:], Wt["w1r"][:], zrt[:, s], start=True, stop=False)
                    nc.tensor.matmul(psr[:], Wt["w1in"][:], zit[:, s], start=False, stop=True)
                    nc.tensor.matmul(psi[:], Wt["w1r"][:], zit[:, s], start=True, stop=False)
                    nc.tensor.matmul(psi[:], Wt["w1i"][:], zrt[:, s], start=False, stop=True)
                    o1rc = spool.tile([C, 512], DT.bfloat16, tag="o1r", name="o1rc")
                    o1ic = spool.tile([C, 512], DT.bfloat16, tag="o1i", name="o1ic")
                    nc.scalar.activation(o1rc[:, :n], psr[:],
                                         mybir.ActivationFunctionType.Relu,
                                         bias=Bt["b1r"][:, 0:1])
                    nc.scalar.activation(o1ic[:, :n], psi[:],
                                         mybir.ActivationFunctionType.Relu,
                                         bias=Bt["b1i"][:, 0:1])
                    psr2 = pspool.tile([C, n], DT.float32, tag="ps")
                    psi2 = pspool.tile([C, n], DT.float32, tag="ps")
                    nc.tensor.matmul(psr2[:], Wt["w2r"][:], o1rc[:, :n], start=True, stop=False)
                    nc.tensor.matmul(psr2[:], Wt["w2in"][:], o1ic[:, :n], start=False, stop=True)
                    nc.tensor.matmul(psi2[:], Wt["w2r"][:], o1ic[:, :n], start=True, stop=False)
                    nc.tensor.matmul(psi2[:], Wt["w2i"][:], o1rc[:, :n], start=False, stop=True)
                    # softshrink(u) with u = psum + b2:
                    #   a1 = relu(psum + (b2 - lam)); a2 = relu(-psum + (-b2 - lam))
                    #   out = a1 - a2
                    for ps, bm, bp, dst in [(psr2, "b2rm", "b2rp", o2r),
                                            (psi2, "b2im", "b2ip", o2i)]:
                        a1 = spool.tile([C, 512], DT.bfloat16, tag="ss1", name="ssa1")
                        a2 = spool.tile([C, 512], DT.bfloat16, tag="ss2", name="ssa2")
                        nc.scalar.activation(a1[:, :n], ps[:],
                                             mybir.ActivationFunctionType.Relu,
                                             bias=Bt[bm][:, 0:1])
                        nc.scalar.activation(a2[:, :n], ps[:],
                                             mybir.ActivationFunctionType.Relu,
                                             bias=Bt[bp][:, 0:1], scale=-1.0)
                        nc.vector.tensor_sub(dst[:, s], a1[:, :n], a2[:, :n])

                # ---- TC: [c,(f,g)] -> [g,(f,c)]; 10-f groups; evict ACT ----
                or_ = papool.tile([128, FC], DT.bfloat16, tag="pa")
                oi_ = papool.tile([128, FC], DT.bfloat16, tag="pa")
                for (src, dst) in [(o2r, or_), (o2i, oi_)]:
                    sv = src[:].rearrange("p (f g) -> p f g", g=128)
                    if noturns:
                        for k in range(6):
                            s = slice(1024 * k, min(1024 * (k + 1), FC))
                            nc.scalar.copy(dst[0:96, s], src[0:96, s])
                        continue
                    for k in range(7):
                        f0 = 10 * k
                        nf = min(10, F - f0)
                        psC = pstpool.tile([128, 960], DT.bfloat16, tag="pst")
                        for j in range(nf):
                            nc.tensor.transpose(
                                psC[:, 96 * j:96 * (j + 1)], sv[:, f0 + j, :],
                                ID[0:96, 0:96]
                            )
                        nc.scalar.copy(dst[:, 96 * f0:96 * (f0 + nf)],
                                       psC[:, :96 * nf])

                # ---- S5: inverse H FFT; evicts DVE ----
                zb = bpool.tile([128, CH], DT.bfloat16, tag="big")
                zbv = zb[:].rearrange("p (c f) -> p f c", f=128)
                for j in range(13):
                    s = slice(480 * j, 480 * (j + 1))
                    pszr = pspool.tile([128, 480], DT.float32, tag="ps")
                    pszi = pspool.tile([128, 480], DT.float32, tag="ps")
                    nc.tensor.matmul(pszr[:], M["dirt"][:], or_[:, s], start=True, stop=False)
                    nc.tensor.matmul(pszr[:], M["diitn"][:], oi_[:, s], start=False, stop=True)
                    nc.tensor.matmul(pszi[:], M["dirt"][:], oi_[:, s], start=True, stop=False)
                    nc.tensor.matmul(pszi[:], M["diit"][:], or_[:, s], start=False, stop=True)
                    f0 = 5 * j
                    nc.vector.tensor_copy(zbv[:, f0:f0 + 5, :], pszr[:])
                    if j == 0:
                        nc.vector.tensor_copy(zbv[:, 65:69, :], pszi[:, 96:480])
                    elif j == 12:
                        nc.vector.tensor_copy(zbv[:, 124:128, :], pszi[:, 0:384])
                    else:
                        nc.vector.tensor_copy(zbv[:, 64 + f0:69 + f0, :], pszi[:])

                # ---- TD: 8-c groups; evict ACT ----
                zp = bpool.tile([128, CH], DT.bfloat16, tag="big")
                zbc = zb[:].rearrange("p (c f) -> p c f", f=128)
                zpv = zp[:].rearrange("p (h c) -> p c h", c=C)
                if noturns:
                    for k in range(CH // 1024):
                        s = slice(1024 * k, 1024 * (k + 1))
                        nc.vector.tensor_copy(zp[:, s], zb[:, s])
                else:
                    for c0 in range(0, C, 8):
                        psD = pstpool.tile([128, 1024], DT.bfloat16, tag="pst")
                        for j in range(8):
                            nc.tensor.transpose(
                                psD[:, 128 * j:128 * (j + 1)], zbc[:, c0 + j, :], ID[:]
                            )
                        nc.scalar.copy(zpv[:, c0:c0 + 8, :], psD[:])

                # ---- S6: inverse W rfft + f32 residual ----
                xrv = xres[b].rearrange("h w c -> w h c")
                orv = out_ext[b].rearrange("h w c -> w h c")
                for j in range(26):
                    lo = 480 * j
                    hi = min(lo + 480, CH)
                    n = hi - lo
                    h0, h1 = lo // C, hi // C
                    ps6 = pspool.tile([128, n], DT.float32, tag="ps")
                    nc.tensor.matmul(ps6[:], M["apack"][:], zp[:, lo:hi],
                                     start=True, stop=True)
                    oc = iopool.tile([128, 480], DT.float32, tag="oc")
                    xr = iopool.tile([128, 480], DT.float32, tag="xr")
                    nc.sync.dma_start(
                        xr[:, :n].rearrange("w (h c) -> w h c", c=C),
                        xrv[:, h0:h1, :])
                    nc.vector.tensor_add(oc[:, :n], ps6[:], xr[:, :n])
                    nc.sync.dma_start(orv[:, h0:h1, :],
                                      oc[:, :n].rearrange("w (h c) -> w h c", c=C))

    nc.compile()
    return nc


def _build_nc_tiny(reps=1):
    """Dispatch-floor calibration kernel: same I/O signature, ~no work."""
    nc = bacc.Bacc(
        "TRN2", target_bir_lowering=False, debug=False, num_devices=N_CORES
    )

    def din(name, shape, dt):
        return nc.dram_tensor(name, shape, dt, kind="ExternalInput")

    x16 = din("x16", [B, W, C, H], DT.bfloat16)
    din("xres", [B, H, W, C], DT.float32)
    for k in ["fpack", "drt", "dit", "ditn", "dirt", "diit", "diitn",
              "apack", "ident"]:
        din(k, [128, 128], DT.bfloat16)
    for k in ["w1r", "w1i", "w1in", "w2r", "w2i", "w2in"]:
        din(k, [C, C], DT.bfloat16)
    for k in ["b1r", "b1i", "b2r", "b2i"]:
        din(k, [C, 1], DT.float32)
    out_ext = nc.dram_tensor("out", [B, H, W, C], DT.float32, kind="ExternalOutput")

    with tile.TileContext(nc) as tc:
        with tc.tile_pool(name="p", bufs=2) as pool:
            for _ in range(reps):
                t = pool.tile([128, 512], DT.bfloat16, name="t")
                nc.sync.dma_start(
                    t[:], x16[0].rearrange("w c h -> w (c h)")[:, 0:512])
    nc.compile()
    return nc
